# revision 25
# baseline (speedup 1.0000x reference)
"""ChebyshevKANLayer on 8 Trainium2 NeuronCores.

y = silu(x) @ Wb + sum_d (x * T_d(xs)) @ Wc[:, :, d]
  xs = per-row rescale of x to [-1, 1]; T_d = Chebyshev polynomials.

Sharding: data-parallel over the batch dim (4096 -> 8 x 512 rows).
Weights replicated (shipped as bf16 to halve the dominant DMA traffic).
No collectives; the host concatenates the shards.

Per-core structure (measured rates: DMA ~326 GB/s, bf16 matmul ~104
ns per [128x128]x[128x512], fp32r ~123 ns):
  - phase A (emitted first so PE/DMA start immediately): silu path --
    sigmoid on ACT, the multiply on gpsimd (writing bf16), 64 matmuls
    into the 8 PSUM accumulators.
  - stats (overlaps phase A): row min/max on DVE from the natural
    shard, tiny affine ops, a 32x32 stream transpose + strided
    SBUF-SBUF DMA gather to form [1, 512] stat rows, then
    gpsimd.partition_broadcast -> [128, 512] broadcast tiles. No PE,
    no PSUM.
  - phase B: per contraction tile, u = 2*xs on DVE, Chebyshev
    recurrence on G_d = x*T_d in fp32 on DVE, per-degree bf16 casts on
    gpsimd, 64 bf16 matmuls.
  - epilogue: PSUM -> SBUF copies (DVE) + output DMA.
"""

import numpy as np

from concourse import bacc, masks, mybir, tile
from concourse.bass_utils import run_bass_kernel_spmd

B, IN, OUT, DEG = 4096, 1024, 1024, 8
NCORES = 8
BS = B // NCORES  # 512 rows per core
KT = IN // 128  # 8 contraction tiles
NB = BS // 128  # 4 batch tiles per core
NO = OUT // 512  # 2 output column tiles

F32 = mybir.dt.float32
BF16 = mybir.dt.bfloat16
ALU = mybir.AluOpType
AF = mybir.ActivationFunctionType
AX = mybir.AxisListType


def _build_kernel(tc, out, xt, xtb, xn, wb, wc, repeat=1):
    nc = tc.nc
    from contextlib import ExitStack

    octx = ExitStack()
    const_pool = octx.enter_context(tc.tile_pool(name="const", bufs=1))
    ident = const_pool.tile([128, 128], F32)
    masks.make_identity(nc, ident[:])
    ones = const_pool.tile([1, 128], F32)
    nc.vector.memset(ones[:], 1.0)
    sb = const_pool.tile([128, BS], F32)  # broadcast of 2*s per column
    tb = const_pool.tile([128, BS], F32)  # broadcast of 2*t per column
    s_row = const_pool.tile([1, BS], F32)
    t_row = const_pool.tile([1, BS], F32)

    with (
        tc.tile_pool(name="psum_acc", bufs=1, space="PSUM") as pacc,
        tc.tile_pool(name="w", bufs=2) as wpool,
        tc.tile_pool(name="g", bufs=2) as gpool,
        tc.tile_pool(name="gb", bufs=2) as gbpool,
        tc.tile_pool(name="xtp", bufs=1) as xtpool,
        tc.tile_pool(name="silu", bufs=2) as slpool,
        tc.tile_pool(name="u", bufs=2) as upool,
        tc.tile_pool(name="o", bufs=2) as opool,
        tc.tile_pool(name="stats", bufs=2) as spool,
    ):
        po = [
            [
                pacc.tile([128, 512], F32, tag=f"po{t}{j}", name=f"po{t}{j}")
                for j in range(NO)
            ]
            for t in range(NB)
        ]
        for rep in range(repeat):
            first = rep == 0
            # --- stats: row min/max -> u = 2*xs = x*s2 + t2 broadcast tiles.
            # The tiny PE-transpose / ones-matmul PSUM outputs alias into the
            # po accumulator banks: the PE runs them (in program order)
            # before the first accumulating matmul, whose start=True reset
            # wipes the scratch values.
            if first:
                for t in range(NB):
                    xnt = spool.tile([128, IN], F32, tag="xnt", name="xnt")
                    nc.sync.dma_start(out=xnt[:], in_=xn[t * 128 : (t + 1) * 128, :])
                    mx = spool.tile([128, 1], F32, tag="mx", name="mx")
                    mn = spool.tile([128, 1], F32, tag="mn", name="mn")
                    nc.vector.tensor_reduce(mx[:], xnt[:], axis=AX.X, op=ALU.max)
                    nc.vector.tensor_reduce(mn[:], xnt[:], axis=AX.X, op=ALU.min)
                    d = spool.tile([128, 1], F32, tag="d", name="d")
                    nc.vector.tensor_tensor(d[:], mx[:], mn[:], ALU.subtract)
                    r = spool.tile([128, 1], F32, tag="r", name="r")
                    nc.vector.reciprocal(r[:], d[:])
                    sc = spool.tile([128, 1], F32, tag="sc", name="sc")
                    nc.vector.tensor_scalar(sc[:], r[:], 4.0, None, ALU.mult)
                    tmp = spool.tile([128, 1], F32, tag="tmp", name="tmp")
                    nc.vector.tensor_tensor(tmp[:], mn[:], sc[:], ALU.mult)
                    tcn = spool.tile([128, 1], F32, tag="tcn", name="tcn")
                    nc.vector.tensor_scalar(
                        tcn[:], tmp[:], -1.0, -2.0, ALU.mult, ALU.add
                    )
                    tsl = slice(t * 128, (t + 1) * 128)
                    nc.tensor.transpose(po[0][0][0:1, tsl], sc[:], ident[:])
                    nc.vector.tensor_copy(s_row[0:1, tsl], po[0][0][0:1, tsl])
                    nc.tensor.transpose(po[0][1][0:1, tsl], tcn[:], ident[:])
                    nc.vector.tensor_copy(t_row[0:1, tsl], po[0][1][0:1, tsl])
                # broadcast the stat rows across all 128 partitions
                nc.tensor.matmul(
                    po[1][0][:], lhsT=ones[:], rhs=s_row[:], start=True, stop=True
                )
                nc.vector.tensor_copy(sb[:], po[1][0][:])
                nc.tensor.matmul(
                    po[1][1][:], lhsT=ones[:], rhs=t_row[:], start=True, stop=True
                )
                nc.vector.tensor_copy(tb[:], po[1][1][:])

            # --- phase A: silu path (independent of row stats) ---
            xtts = []
            xbts = []
            for k in range(KT):
                ksl = slice(k * 128, (k + 1) * 128)
                xtt = xtpool.tile([128, BS], F32, tag=f"xtt{k}", name=f"xtt{k}")
                xtts.append(xtt)
                nc.sync.dma_start(out=xtt[:], in_=xt[ksl, :])
                xbt = xtpool.tile([128, BS], BF16, tag=f"xbt{k}", name=f"xbt{k}")
                xbts.append(xbt)
                nc.sync.dma_start(out=xbt[:], in_=xtb[ksl, :])
                wbt = wpool.tile([128, OUT], BF16, tag="wbt", name="wbt")
                nc.sync.dma_start(out=wbt[:], in_=wb[ksl, :])
                sl = slpool.tile([128, BS], BF16, tag="sl", name="sl")
                sigt = slpool.tile([128, BS], F32, tag="sigt", name="sigt")
                # silu = x*sigmoid(x); multiply on gpsimd, rounding to bf16
                nc.scalar.activation(sigt[:], xtt[:], AF.Sigmoid)
                nc.gpsimd.tensor_tensor(sl[:], sigt[:], xtt[:], ALU.mult)
                for t in range(NB):
                    lhs = sl[:, t * 128 : (t + 1) * 128]
                    for j in range(NO):
                        rhs = wbt[:, j * 512 : (j + 1) * 512]
                        nc.tensor.matmul(
                            po[t][j][:],
                            lhsT=lhs,
                            rhs=rhs,
                            start=(k == 0),
                            stop=False,
                        )

            # --- phase B: chebyshev paths ---
            for k in range(KT):
                ksl = slice(k * 128, (k + 1) * 128)
                xtt = xtts[k]
                xbt = xbts[k]
                wall = wpool.tile([128, DEG * OUT], BF16, tag="wall", name="wall")
                for dg in range(DEG):
                    nc.sync.dma_start(
                        out=wall[:, dg * OUT : (dg + 1) * OUT], in_=wc[dg, ksl, :]
                    )
                gall = gpool.tile([128, (DEG - 1) * BS], F32, tag="gall", name="gall")
                gball = gbpool.tile(
                    [128, (DEG - 1) * BS], BF16, tag="gball", name="gball"
                )

                def Gs(i):
                    # fp32 recurrence slots G_1..G_7
                    return gall[:, (i - 1) * BS : i * BS]

                def Gb(i):
                    # bf16 matmul operand slots G_1..G_7
                    return gball[:, (i - 1) * BS : i * BS]

                ut = upool.tile([128, BS], F32, tag="ut", name="ut")
                nc.vector.tensor_tensor(ut[:], xtt[:], sb[:], ALU.mult)
                nc.vector.tensor_tensor(ut[:], ut[:], tb[:], ALU.add)
                # G_1 = x * xs = (x * 0.5) * u
                nc.vector.scalar_tensor_tensor(
                    Gs(1), in0=xtt[:], scalar=0.5, in1=ut[:], op0=ALU.mult, op1=ALU.mult
                )
                nc.gpsimd.tensor_copy(Gb(1), Gs(1))
                for dg in range(2, DEG):
                    tmpd = upool.tile([128, BS], F32, tag="tmpd", name="tmpd")
                    nc.vector.tensor_tensor(tmpd[:], ut[:], Gs(dg - 1), ALU.mult)
                    prev2 = xtt[:] if dg == 2 else Gs(dg - 2)
                    nc.vector.tensor_tensor(Gs(dg), tmpd[:], prev2, ALU.subtract)
                    nc.gpsimd.tensor_copy(Gb(dg), Gs(dg))

                gstat = [xbt] + [Gb(i) for i in range(1, DEG)]
                for t in range(NB):
                    for m in range(DEG):
                        lhs = gstat[m][:, t * 128 : (t + 1) * 128]
                        for j in range(NO):
                            rhs = wall[:, m * OUT + j * 512 : m * OUT + (j + 1) * 512]
                            nc.tensor.matmul(
                                po[t][j][:],
                                lhsT=lhs,
                                rhs=rhs,
                                start=False,
                                stop=(k == KT - 1 and m == DEG - 1),
                            )
        for t in range(NB):
            for j in range(NO):
                ot = opool.tile([128, 512], F32, tag="ot", name="ot")
                nc.vector.tensor_copy(ot[:], po[t][j][:])
                nc.sync.dma_start(
                    out=out[t * 128 : (t + 1) * 128, j * 512 : (j + 1) * 512],
                    in_=ot[:],
                )
    octx.close()


_NC_CACHE = {}


def build_nc(repeat=1):
    if repeat in _NC_CACHE:
        return _NC_CACHE[repeat]
    nc = bacc.Bacc(
        "TRN2", target_bir_lowering=False, debug=False, num_devices=NCORES
    )
    xt = nc.dram_tensor("xt", [IN, BS], F32, kind="ExternalInput").ap()
    xtb = nc.dram_tensor("xtb", [IN, BS], BF16, kind="ExternalInput").ap()
    xn = nc.dram_tensor("xn", [BS, IN], F32, kind="ExternalInput").ap()
    wb = nc.dram_tensor("wb", [IN, OUT], BF16, kind="ExternalInput").ap()
    wc = nc.dram_tensor("wc", [DEG, IN, OUT], BF16, kind="ExternalInput").ap()
    out = nc.dram_tensor("out", [BS, OUT], F32, kind="ExternalOutput").ap()
    with tile.TileContext(nc) as tc:
        _build_kernel(tc, out, xt, xtb, xn, wb, wc, repeat=repeat)
    nc.compile()
    _NC_CACHE[repeat] = nc
    return nc


def make_in_maps(x, base_weight, cheb_weight):
    import ml_dtypes

    x = np.ascontiguousarray(np.asarray(x, dtype=np.float32))
    wb = np.asarray(base_weight, dtype=np.float32).astype(ml_dtypes.bfloat16)
    wc = np.ascontiguousarray(
        np.asarray(cheb_weight, dtype=np.float32)
        .transpose(2, 0, 1)
        .astype(ml_dtypes.bfloat16)
    )
    in_maps = []
    for c in range(NCORES):
        shard = x[c * BS : (c + 1) * BS]
        shard_t = np.ascontiguousarray(shard.T)
        in_maps.append(
            {
                "xt": shard_t,
                "xtb": shard_t.astype(ml_dtypes.bfloat16),
                "xn": shard,
                "wb": wb,
                "wc": wc,
            }
        )
    return in_maps


def kernel(x, base_weight, cheb_weight, degree=DEG, **_):
    assert int(degree) == DEG
    nc = build_nc()
    in_maps = make_in_maps(x, base_weight, cheb_weight)
    res = run_bass_kernel_spmd(nc, in_maps, list(range(NCORES)))
    return np.concatenate([r["out"] for r in res.results], axis=0)


# revision 26
# speedup vs baseline: 1.0086x; 1.0086x over previous
"""ChebyshevKANLayer on 8 Trainium2 NeuronCores.

y = silu(x) @ Wb + sum_d (x * T_d(xs)) @ Wc[:, :, d]
  xs = per-row rescale of x to [-1, 1]; T_d = Chebyshev polynomials.

Sharding: data-parallel over the batch dim (4096 -> 8 x 512 rows).
Weights replicated (shipped as bf16 to halve the dominant DMA traffic).
No collectives; the host concatenates the shards.

Per-core structure (measured rates: DMA ~326 GB/s, bf16 matmul ~104
ns per [128x128]x[128x512], fp32r ~123 ns):
  - phase A (emitted first so PE/DMA start immediately): silu path --
    sigmoid on ACT, the multiply on gpsimd (writing bf16), 64 matmuls
    into the 8 PSUM accumulators.
  - stats (overlaps phase A): row min/max on DVE from the natural
    shard, tiny affine ops, a 32x32 stream transpose + strided
    SBUF-SBUF DMA gather to form [1, 512] stat rows, then
    gpsimd.partition_broadcast -> [128, 512] broadcast tiles. No PE,
    no PSUM.
  - phase B: per contraction tile, u = 2*xs on DVE, Chebyshev
    recurrence on G_d = x*T_d in fp32 on DVE, per-degree bf16 casts on
    gpsimd, 64 bf16 matmuls.
  - epilogue: PSUM -> SBUF copies (DVE) + output DMA.
"""

import numpy as np

from concourse import bacc, masks, mybir, tile
from concourse.bass_utils import run_bass_kernel_spmd

B, IN, OUT, DEG = 4096, 1024, 1024, 8
NCORES = 8
BS = B // NCORES  # 512 rows per core
KT = IN // 128  # 8 contraction tiles
NB = BS // 128  # 4 batch tiles per core
NO = OUT // 512  # 2 output column tiles

F32 = mybir.dt.float32
BF16 = mybir.dt.bfloat16
ALU = mybir.AluOpType
AF = mybir.ActivationFunctionType
AX = mybir.AxisListType


def _build_kernel(tc, out, xt, xtb, xn, wb, wc, repeat=1):
    nc = tc.nc
    from contextlib import ExitStack

    octx = ExitStack()
    const_pool = octx.enter_context(tc.tile_pool(name="const", bufs=1))
    ident = const_pool.tile([128, 128], F32)
    masks.make_identity(nc, ident[:])
    ones = const_pool.tile([1, 128], F32)
    nc.vector.memset(ones[:], 1.0)
    sb = const_pool.tile([128, BS], F32)  # broadcast of 2*s per column
    tb = const_pool.tile([128, BS], F32)  # broadcast of 2*t per column
    s_row = const_pool.tile([1, BS], F32)
    t_row = const_pool.tile([1, BS], F32)

    with (
        tc.tile_pool(name="psum_acc", bufs=1, space="PSUM") as pacc,
        tc.tile_pool(name="w", bufs=2) as wpool,
        tc.tile_pool(name="g", bufs=2) as gpool,
        tc.tile_pool(name="gb", bufs=2) as gbpool,
        tc.tile_pool(name="xtp", bufs=1) as xtpool,
        tc.tile_pool(name="silu", bufs=2) as slpool,
        tc.tile_pool(name="u", bufs=2) as upool,
        tc.tile_pool(name="o", bufs=2) as opool,
        tc.tile_pool(name="stats", bufs=2) as spool,
    ):
        po = [
            [
                pacc.tile([128, 512], F32, tag=f"po{t}{j}", name=f"po{t}{j}")
                for j in range(NO)
            ]
            for t in range(NB)
        ]
        for rep in range(repeat):
            first = rep == 0
            # --- stats: row min/max -> u = 2*xs = x*s2 + t2 broadcast tiles.
            # The tiny PE-transpose / ones-matmul PSUM outputs alias into the
            # po accumulator banks: the PE runs them (in program order)
            # before the first accumulating matmul, whose start=True reset
            # wipes the scratch values.
            if first:
                for t in range(NB):
                    xnt = spool.tile([128, IN], F32, tag="xnt", name="xnt")
                    nc.sync.dma_start(out=xnt[:], in_=xn[t * 128 : (t + 1) * 128, :])
                    mx = spool.tile([128, 1], F32, tag="mx", name="mx")
                    mn = spool.tile([128, 1], F32, tag="mn", name="mn")
                    nc.vector.tensor_reduce(mx[:], xnt[:], axis=AX.X, op=ALU.max)
                    nc.vector.tensor_reduce(mn[:], xnt[:], axis=AX.X, op=ALU.min)
                    d = spool.tile([128, 1], F32, tag="d", name="d")
                    nc.vector.tensor_tensor(d[:], mx[:], mn[:], ALU.subtract)
                    r = spool.tile([128, 1], F32, tag="r", name="r")
                    nc.vector.reciprocal(r[:], d[:])
                    sc = spool.tile([128, 1], F32, tag="sc", name="sc")
                    nc.vector.tensor_scalar(sc[:], r[:], 4.0, None, ALU.mult)
                    tmp = spool.tile([128, 1], F32, tag="tmp", name="tmp")
                    nc.vector.tensor_tensor(tmp[:], mn[:], sc[:], ALU.mult)
                    tcn = spool.tile([128, 1], F32, tag="tcn", name="tcn")
                    nc.vector.tensor_scalar(
                        tcn[:], tmp[:], -1.0, -2.0, ALU.mult, ALU.add
                    )
                    tsl = slice(t * 128, (t + 1) * 128)
                    nc.tensor.transpose(po[0][0][0:1, tsl], sc[:], ident[:])
                    nc.vector.tensor_copy(s_row[0:1, tsl], po[0][0][0:1, tsl])
                    nc.tensor.transpose(po[0][1][0:1, tsl], tcn[:], ident[:])
                    nc.vector.tensor_copy(t_row[0:1, tsl], po[0][1][0:1, tsl])
                # broadcast the stat rows across all 128 partitions
                nc.tensor.matmul(
                    po[1][0][:], lhsT=ones[:], rhs=s_row[:], start=True, stop=True
                )
                nc.vector.tensor_copy(sb[:], po[1][0][:])
                nc.tensor.matmul(
                    po[1][1][:], lhsT=ones[:], rhs=t_row[:], start=True, stop=True
                )
                nc.vector.tensor_copy(tb[:], po[1][1][:])

            # --- phase A: silu path (independent of row stats) ---
            xtts = []
            xbts = []
            for k in range(KT):
                ksl = slice(k * 128, (k + 1) * 128)
                xtt = xtpool.tile([128, BS], F32, tag=f"xtt{k}", name=f"xtt{k}")
                xtts.append(xtt)
                nc.sync.dma_start(out=xtt[:], in_=xt[ksl, :])
                xbt = xtpool.tile([128, BS], BF16, tag=f"xbt{k}", name=f"xbt{k}")
                xbts.append(xbt)
                nc.sync.dma_start(out=xbt[:], in_=xtb[ksl, :])
                wbt = wpool.tile([128, OUT], BF16, tag="wbt", name="wbt")
                nc.sync.dma_start(out=wbt[:], in_=wb[ksl, :])
                sl = slpool.tile([128, BS], BF16, tag="sl", name="sl")
                sigt = slpool.tile([128, BS], F32, tag="sigt", name="sigt")
                # silu = x*sigmoid(x); multiply on gpsimd, rounding to bf16
                nc.scalar.activation(sigt[:], xtt[:], AF.Sigmoid)
                nc.gpsimd.tensor_tensor(sl[:], sigt[:], xtt[:], ALU.mult)
                for t in range(NB):
                    lhs = sl[:, t * 128 : (t + 1) * 128]
                    for j in range(NO):
                        rhs = wbt[:, j * 512 : (j + 1) * 512]
                        nc.tensor.matmul(
                            po[t][j][:],
                            lhsT=lhs,
                            rhs=rhs,
                            start=(k == 0),
                            stop=False,
                        )

            # --- phase B: chebyshev paths ---
            # fp32 recurrence chain on DVE (plus a few mults on gpsimd);
            # bf16 matmul operands come from duplicate subtracts with bf16
            # output on gpsimd (cheaper there than fp32 ops); fp32 G_7 is
            # never consumed, so its fp32 subtract is skipped.
            GP_MULTS = {3, 5, 7}  # chain mults placed on gpsimd
            for k in range(KT):
                ksl = slice(k * 128, (k + 1) * 128)
                xtt = xtts[k]
                xbt = xbts[k]
                wall = wpool.tile([128, DEG * OUT], BF16, tag="wall", name="wall")
                for dg in range(DEG):
                    nc.sync.dma_start(
                        out=wall[:, dg * OUT : (dg + 1) * OUT], in_=wc[dg, ksl, :]
                    )
                gall = gpool.tile([128, (DEG - 2) * BS], F32, tag="gall", name="gall")
                gball = gbpool.tile(
                    [128, (DEG - 1) * BS], BF16, tag="gball", name="gball"
                )

                def Gs(i):
                    # fp32 recurrence slots G_1..G_6
                    return gall[:, (i - 1) * BS : i * BS]

                def Gb(i):
                    # bf16 matmul operand slots G_1..G_7
                    return gball[:, (i - 1) * BS : i * BS]

                ut = upool.tile([128, BS], F32, tag="ut", name="ut")
                nc.vector.tensor_tensor(ut[:], xtt[:], sb[:], ALU.mult)
                nc.vector.tensor_tensor(ut[:], ut[:], tb[:], ALU.add)
                # G_1 = x * xs = (x * 0.5) * u
                nc.vector.scalar_tensor_tensor(
                    Gs(1), in0=xtt[:], scalar=0.5, in1=ut[:], op0=ALU.mult, op1=ALU.mult
                )
                nc.scalar.activation(Gb(1), Gs(1), AF.Copy)
                for dg in range(2, DEG):
                    tmpd = upool.tile([128, BS], F32, tag=f"tmpd{dg}", name="tmpd")
                    meng = nc.gpsimd if dg in GP_MULTS else nc.vector
                    meng.tensor_tensor(tmpd[:], ut[:], Gs(dg - 1), ALU.mult)
                    prev2 = xtt[:] if dg == 2 else Gs(dg - 2)
                    if dg < DEG - 1:
                        nc.vector.tensor_tensor(Gs(dg), tmpd[:], prev2, ALU.subtract)
                    nc.gpsimd.tensor_tensor(Gb(dg), tmpd[:], prev2, ALU.subtract)

                gstat = [xbt] + [Gb(i) for i in range(1, DEG)]
                for t in range(NB):
                    for m in range(DEG):
                        lhs = gstat[m][:, t * 128 : (t + 1) * 128]
                        for j in range(NO):
                            rhs = wall[:, m * OUT + j * 512 : m * OUT + (j + 1) * 512]
                            nc.tensor.matmul(
                                po[t][j][:],
                                lhsT=lhs,
                                rhs=rhs,
                                start=False,
                                stop=(k == KT - 1 and m == DEG - 1),
                            )
        for t in range(NB):
            for j in range(NO):
                ot = opool.tile([128, 512], F32, tag="ot", name="ot")
                nc.scalar.activation(ot[:], po[t][j][:], AF.Copy)
                nc.sync.dma_start(
                    out=out[t * 128 : (t + 1) * 128, j * 512 : (j + 1) * 512],
                    in_=ot[:],
                )
    octx.close()


_NC_CACHE = {}


def build_nc(repeat=1):
    if repeat in _NC_CACHE:
        return _NC_CACHE[repeat]
    nc = bacc.Bacc(
        "TRN2", target_bir_lowering=False, debug=False, num_devices=NCORES
    )
    xt = nc.dram_tensor("xt", [IN, BS], F32, kind="ExternalInput").ap()
    xtb = nc.dram_tensor("xtb", [IN, BS], BF16, kind="ExternalInput").ap()
    xn = nc.dram_tensor("xn", [BS, IN], F32, kind="ExternalInput").ap()
    wb = nc.dram_tensor("wb", [IN, OUT], BF16, kind="ExternalInput").ap()
    wc = nc.dram_tensor("wc", [DEG, IN, OUT], BF16, kind="ExternalInput").ap()
    out = nc.dram_tensor("out", [BS, OUT], F32, kind="ExternalOutput").ap()
    with tile.TileContext(nc) as tc:
        _build_kernel(tc, out, xt, xtb, xn, wb, wc, repeat=repeat)
    nc.compile()
    _NC_CACHE[repeat] = nc
    return nc


def make_in_maps(x, base_weight, cheb_weight):
    import ml_dtypes

    x = np.ascontiguousarray(np.asarray(x, dtype=np.float32))
    wb = np.asarray(base_weight, dtype=np.float32).astype(ml_dtypes.bfloat16)
    wc = np.ascontiguousarray(
        np.asarray(cheb_weight, dtype=np.float32)
        .transpose(2, 0, 1)
        .astype(ml_dtypes.bfloat16)
    )
    in_maps = []
    for c in range(NCORES):
        shard = x[c * BS : (c + 1) * BS]
        shard_t = np.ascontiguousarray(shard.T)
        in_maps.append(
            {
                "xt": shard_t,
                "xtb": shard_t.astype(ml_dtypes.bfloat16),
                "xn": shard,
                "wb": wb,
                "wc": wc,
            }
        )
    return in_maps


def kernel(x, base_weight, cheb_weight, degree=DEG, **_):
    assert int(degree) == DEG
    nc = build_nc()
    in_maps = make_in_maps(x, base_weight, cheb_weight)
    res = run_bass_kernel_spmd(nc, in_maps, list(range(NCORES)))
    return np.concatenate([r["out"] for r in res.results], axis=0)


# revision 31
# speedup vs baseline: 1.6031x; 1.5895x over previous
"""ChebyshevKANLayer on 8 Trainium2 NeuronCores.

y = silu(x) @ Wb + sum_d (x * T_d(xs)) @ Wc[:, :, d]
  xs = per-row rescale of x to [-1, 1]; T_d = Chebyshev polynomials.

Sharding: data-parallel over the batch dim (4096 -> 8 x 512 rows).
Weights replicated (shipped as bf16 to halve the dominant DMA traffic).
No collectives; the host concatenates the shards.

Per-core structure (measured rates: DMA ~326 GB/s, bf16 matmul ~104
ns per [128x128]x[128x512], fp32r ~123 ns):
  - phase A (emitted first so PE/DMA start immediately): silu path --
    sigmoid on ACT, the multiply on gpsimd (writing bf16), 64 matmuls
    into the 8 PSUM accumulators.
  - stats (overlaps phase A): row min/max on DVE from the natural
    shard, tiny affine ops, a 32x32 stream transpose + strided
    SBUF-SBUF DMA gather to form [1, 512] stat rows, then
    gpsimd.partition_broadcast -> [128, 512] broadcast tiles. No PE,
    no PSUM.
  - phase B: per contraction tile, u = 2*xs on DVE, Chebyshev
    recurrence on G_d = x*T_d in fp32 on DVE, per-degree bf16 casts on
    gpsimd, 64 bf16 matmuls.
  - epilogue: PSUM -> SBUF copies (DVE) + output DMA.
"""

import numpy as np

from concourse import bacc, masks, mybir, tile
from concourse.bass_utils import run_bass_kernel_spmd

B, IN, OUT, DEG = 4096, 1024, 1024, 8
NCORES = 8
BS = B // NCORES  # 512 rows per core
KT = IN // 128  # 8 contraction tiles
NB = BS // 128  # 4 batch tiles per core
NO = OUT // 512  # 2 output column tiles

F32 = mybir.dt.float32
BF16 = mybir.dt.bfloat16
ALU = mybir.AluOpType
AF = mybir.ActivationFunctionType
AX = mybir.AxisListType


def _build_kernel(tc, out, xt, xtb, xn, wb, wc, repeat=1):
    nc = tc.nc
    from contextlib import ExitStack

    octx = ExitStack()
    const_pool = octx.enter_context(tc.tile_pool(name="const", bufs=1))
    ident = const_pool.tile([128, 128], F32)
    masks.make_identity(nc, ident[:])
    ones = const_pool.tile([1, 128], F32)
    nc.vector.memset(ones[:], 1.0)
    sb = const_pool.tile([128, BS], F32)  # broadcast of 2*s per column
    tb = const_pool.tile([128, BS], F32)  # broadcast of 2*t per column
    s_row = const_pool.tile([1, BS], F32)
    t_row = const_pool.tile([1, BS], F32)

    with (
        tc.tile_pool(name="psum_acc", bufs=1, space="PSUM") as pacc,
        tc.tile_pool(name="w", bufs=2) as wpool,
        tc.tile_pool(name="g", bufs=2) as gpool,
        tc.tile_pool(name="gb", bufs=2) as gbpool,
        tc.tile_pool(name="xtp", bufs=1) as xtpool,
        tc.tile_pool(name="silu", bufs=2) as slpool,
        tc.tile_pool(name="u", bufs=2) as upool,
        tc.tile_pool(name="o", bufs=2) as opool,
        tc.tile_pool(name="stats", bufs=2) as spool,
    ):
        po = [
            [
                pacc.tile([128, 512], F32, tag=f"po{t}{j}", name=f"po{t}{j}")
                for j in range(NO)
            ]
            for t in range(NB)
        ]
        for rep in range(repeat):
            first = rep == 0
            # --- stats: row min/max -> u = 2*xs = x*s2 + t2 broadcast tiles.
            # The tiny PE-transpose / ones-matmul PSUM outputs alias into the
            # po accumulator banks: the PE runs them (in program order)
            # before the first accumulating matmul, whose start=True reset
            # wipes the scratch values.
            if first:
                for t in range(NB):
                    xnt = spool.tile([128, IN], F32, tag="xnt", name="xnt")
                    nc.sync.dma_start(out=xnt[:], in_=xn[t * 128 : (t + 1) * 128, :])
                    mx = spool.tile([128, 1], F32, tag="mx", name="mx")
                    mn = spool.tile([128, 1], F32, tag="mn", name="mn")
                    nc.vector.tensor_reduce(mx[:], xnt[:], axis=AX.X, op=ALU.max)
                    nc.vector.tensor_reduce(mn[:], xnt[:], axis=AX.X, op=ALU.min)
                    d = spool.tile([128, 1], F32, tag="d", name="d")
                    nc.vector.tensor_tensor(d[:], mx[:], mn[:], ALU.subtract)
                    r = spool.tile([128, 1], F32, tag="r", name="r")
                    nc.vector.reciprocal(r[:], d[:])
                    sc = spool.tile([128, 1], F32, tag="sc", name="sc")
                    nc.vector.tensor_scalar(sc[:], r[:], 4.0, None, ALU.mult)
                    tmp = spool.tile([128, 1], F32, tag="tmp", name="tmp")
                    nc.vector.tensor_tensor(tmp[:], mn[:], sc[:], ALU.mult)
                    tcn = spool.tile([128, 1], F32, tag="tcn", name="tcn")
                    nc.vector.tensor_scalar(
                        tcn[:], tmp[:], -1.0, -2.0, ALU.mult, ALU.add
                    )
                    tsl = slice(t * 128, (t + 1) * 128)
                    nc.tensor.transpose(po[0][0][0:1, tsl], sc[:], ident[:])
                    nc.vector.tensor_copy(s_row[0:1, tsl], po[0][0][0:1, tsl])
                    nc.tensor.transpose(po[0][1][0:1, tsl], tcn[:], ident[:])
                    nc.vector.tensor_copy(t_row[0:1, tsl], po[0][1][0:1, tsl])
                # broadcast the stat rows across all 128 partitions
                nc.tensor.matmul(
                    po[1][0][:], lhsT=ones[:], rhs=s_row[:], start=True, stop=True
                )
                nc.vector.tensor_copy(sb[:], po[1][0][:])
                nc.tensor.matmul(
                    po[1][1][:], lhsT=ones[:], rhs=t_row[:], start=True, stop=True
                )
                nc.vector.tensor_copy(tb[:], po[1][1][:])

            # --- phase A: silu path (independent of row stats) ---
            xtts = []
            xbts = []
            for k in range(KT):
                ksl = slice(k * 128, (k + 1) * 128)
                xtt = xtpool.tile([128, BS], F32, tag=f"xtt{k}", name=f"xtt{k}")
                xtts.append(xtt)
                nc.sync.dma_start(out=xtt[:], in_=xt[ksl, :])
                xbt = xtpool.tile([128, BS], BF16, tag=f"xbt{k}", name=f"xbt{k}")
                xbts.append(xbt)
                nc.sync.dma_start(out=xbt[:], in_=xtb[ksl, :])
                wbt = wpool.tile([128, OUT], BF16, tag="wbt", name="wbt")
                nc.sync.dma_start(out=wbt[:], in_=wb[ksl, :])
                sl = slpool.tile([128, BS], BF16, tag="sl", name="sl")
                sigt = slpool.tile([128, BS], F32, tag="sigt", name="sigt")
                # silu = x*sigmoid(x), rounding to bf16
                nc.scalar.activation(sigt[:], xtt[:], AF.Sigmoid)
                nc.vector.tensor_tensor(sl[:], sigt[:], xtt[:], ALU.mult)
                for t in range(NB):
                    lhs = sl[:, t * 128 : (t + 1) * 128]
                    for j in range(NO):
                        rhs = wbt[:, j * 512 : (j + 1) * 512]
                        nc.tensor.matmul(
                            po[t][j][:],
                            lhsT=lhs,
                            rhs=rhs,
                            start=(k == 0),
                            stop=False,
                        )

            # --- phase B: chebyshev paths ---
            # Everything on DVE + ACT: in-context gpsimd ops measured ~2.5us
            # each (sem-wait + software dispatch), so the whole chain, the
            # cheap 312ns bf16 casts, and the silu multiply stay on DVE.
            GP_MULTS = ()  # chain mults placed on gpsimd (none)
            for k in range(KT):
                ksl = slice(k * 128, (k + 1) * 128)
                xtt = xtts[k]
                xbt = xbts[k]
                wall = wpool.tile([128, DEG * OUT], BF16, tag="wall", name="wall")
                for dg in range(DEG):
                    nc.sync.dma_start(
                        out=wall[:, dg * OUT : (dg + 1) * OUT], in_=wc[dg, ksl, :]
                    )
                gall = gpool.tile([128, (DEG - 1) * BS], F32, tag="gall", name="gall")
                gball = gbpool.tile(
                    [128, (DEG - 1) * BS], BF16, tag="gball", name="gball"
                )

                def Gs(i):
                    # fp32 recurrence slots G_1..G_7
                    return gall[:, (i - 1) * BS : i * BS]

                def Gb(i):
                    # bf16 matmul operand slots G_1..G_7
                    return gball[:, (i - 1) * BS : i * BS]

                ut = upool.tile([128, BS], F32, tag="ut", name="ut")
                nc.vector.tensor_tensor(ut[:], xtt[:], sb[:], ALU.mult)
                nc.vector.tensor_tensor(ut[:], ut[:], tb[:], ALU.add)
                # G_1 = x * xs = (x * 0.5) * u
                nc.vector.scalar_tensor_tensor(
                    Gs(1), in0=xtt[:], scalar=0.5, in1=ut[:], op0=ALU.mult, op1=ALU.mult
                )
                nc.scalar.activation(Gb(1), Gs(1), AF.Copy)
                for dg in range(2, DEG):
                    tmpd = upool.tile([128, BS], F32, tag=f"tmpd{dg}", name="tmpd")
                    meng = nc.gpsimd if dg in GP_MULTS else nc.vector
                    meng.tensor_tensor(tmpd[:], ut[:], Gs(dg - 1), ALU.mult)
                    prev2 = xtt[:] if dg == 2 else Gs(dg - 2)
                    nc.vector.tensor_tensor(Gs(dg), tmpd[:], prev2, ALU.subtract)
                    nc.vector.tensor_copy(Gb(dg), Gs(dg))

                gstat = [xbt] + [Gb(i) for i in range(1, DEG)]
                for t in range(NB):
                    for m in range(DEG):
                        lhs = gstat[m][:, t * 128 : (t + 1) * 128]
                        for j in range(NO):
                            rhs = wall[:, m * OUT + j * 512 : m * OUT + (j + 1) * 512]
                            nc.tensor.matmul(
                                po[t][j][:],
                                lhsT=lhs,
                                rhs=rhs,
                                start=False,
                                stop=(k == KT - 1 and m == DEG - 1),
                            )
        for t in range(NB):
            for j in range(NO):
                ot = opool.tile([128, 512], F32, tag="ot", name="ot")
                nc.scalar.activation(ot[:], po[t][j][:], AF.Copy)
                nc.sync.dma_start(
                    out=out[t * 128 : (t + 1) * 128, j * 512 : (j + 1) * 512],
                    in_=ot[:],
                )
    octx.close()


_NC_CACHE = {}


def build_nc(repeat=1):
    if repeat in _NC_CACHE:
        return _NC_CACHE[repeat]
    nc = bacc.Bacc(
        "TRN2", target_bir_lowering=False, debug=False, num_devices=NCORES
    )
    xt = nc.dram_tensor("xt", [IN, BS], F32, kind="ExternalInput").ap()
    xtb = nc.dram_tensor("xtb", [IN, BS], BF16, kind="ExternalInput").ap()
    xn = nc.dram_tensor("xn", [BS, IN], F32, kind="ExternalInput").ap()
    wb = nc.dram_tensor("wb", [IN, OUT], BF16, kind="ExternalInput").ap()
    wc = nc.dram_tensor("wc", [DEG, IN, OUT], BF16, kind="ExternalInput").ap()
    out = nc.dram_tensor("out", [BS, OUT], F32, kind="ExternalOutput").ap()
    with tile.TileContext(nc) as tc:
        _build_kernel(tc, out, xt, xtb, xn, wb, wc, repeat=repeat)
    nc.compile()
    _NC_CACHE[repeat] = nc
    return nc


def make_in_maps(x, base_weight, cheb_weight):
    import ml_dtypes

    x = np.ascontiguousarray(np.asarray(x, dtype=np.float32))
    wb = np.asarray(base_weight, dtype=np.float32).astype(ml_dtypes.bfloat16)
    wc = np.ascontiguousarray(
        np.asarray(cheb_weight, dtype=np.float32)
        .transpose(2, 0, 1)
        .astype(ml_dtypes.bfloat16)
    )
    in_maps = []
    for c in range(NCORES):
        shard = x[c * BS : (c + 1) * BS]
        shard_t = np.ascontiguousarray(shard.T)
        in_maps.append(
            {
                "xt": shard_t,
                "xtb": shard_t.astype(ml_dtypes.bfloat16),
                "xn": shard,
                "wb": wb,
                "wc": wc,
            }
        )
    return in_maps


def kernel(x, base_weight, cheb_weight, degree=DEG, **_):
    assert int(degree) == DEG
    nc = build_nc()
    in_maps = make_in_maps(x, base_weight, cheb_weight)
    res = run_bass_kernel_spmd(nc, in_maps, list(range(NCORES)))
    return np.concatenate([r["out"] for r in res.results], axis=0)
